# revision 9
# baseline (speedup 1.0000x reference)
# DenseGATv2Conv Trainium2 kernel (v3).
#
# Math (per batch b):
#   xl = x @ W_l + b_l ; xr = x @ W_r + b_r            [N, H*C]
#   alpha[i,j,h] = sum_c att[h,c] * leaky_relu(xl[j,hc] + xr[i,hc], 0.2)
#   S = softmax_j(alpha masked by adj(+self loops))
#   out[i,hc] = sum_j S[i,j,h] * xr[j,hc] + bias
#
# Identities used on device:
#   leaky_relu(z) = 0.2*z + 0.8*relu(z)
#   alpha[i,j,h] = 0.2*sl[j,h] + 0.2*sr[i,h] + 0.8*sum_c att[h,c]*relu(xl[j,hc]+xr[i,hc])
# exp(0.2*sr[i,h]) cancels in the softmax; exp(0.2*sl[j,h]) (= esl) is folded
# multiplicatively into the aggregation operand; the output bias is folded
# into the aggregation operand too, via (num + bias*den)/den.
#
# v3 changes vs v2:
#  * 9 of 16 pairs per super run in fp8: relu data is produced directly in
#    fp8e4m3 (DVE / Act / GpSimd share the production load) and consumed by
#    DoubleRow matmuls that pack TWO pairs per pass (2 k-tiles, disjoint
#    stationary columns), costing 0.5 PE cycles/row.  The fp8 rounding of
#    0.8*att is compensated exactly by scaling the relu production by
#    ratio[hc] = 0.8*att/fp8(0.8*att) (folded into the Act scale operand /
#    a prescaled copy of xl^T), so only the relu-value quantization noise
#    remains (~1.2e-2 rel).
#  * The adjacency mask is applied as a -15 additive bias inside the score
#    PSUM accumulation via one more fp8 DoubleRow matmul per half (moving =
#    -15*(1-adj) host-prepared fp8, stationary = 0/1 dest-row selector).
#    This removes the post-exp DVE multiply and 1.75MB of DMA.
#  * The remaining 7 pairs per super stay fp16 (DVE production + fp16
#    matmuls with tile-position banding) to keep the overall rel error
#    ~1.2e-2, under the 2e-2 gate.
#
# Sharding: 8 cores = (batch b in 0..1) x (4 blocks of 256 destination rows).

import numpy as np

B, N, F, H, C = 2, 1024, 128, 4, 16
HC = H * C
NCORES = 8
NI = 256          # destination rows per core
NSUP = 8          # supers of 16 pairs (32 dest rows) each

# fp8 duo passes: [(q,v),(q,v+1)] share one DoubleRow matmul per half.
FP8_DUOS = [((0, 0), (0, 1)), ((0, 2), (0, 3)),
            ((1, 0), (1, 1)), ((1, 2), (1, 3))]
FP8_SOLO = (2, 0)
F16_PAIRS = [(2, 1), (2, 2), (2, 3), (3, 0), (3, 1), (3, 2), (3, 3)]

# probe modes: "mixed" (default), "allf16", "dualsolo" (each duo as 2 solo
# DR passes), "plainfp8" (non-DR fp8 matmuls)
import os as _os
MODE = _os.environ.get("V3_MODE", "mixed")

ALL_FP8 = [p for duo in FP8_DUOS for p in duo] + [FP8_SOLO]


def _pass_list():
    # list of (u0_pair, u1_pair_or_None) stationary pass definitions
    if MODE == "mixed":
        return list(FP8_DUOS) + [(FP8_SOLO, None)]
    if MODE in ("dualsolo", "plainfp8"):
        return [(p, None) for p in ALL_FP8]
    return []   # allf16


def _fp8_engine(sup, q, v):
    # production engine per fp8 pair, balancing DVE/Act/Pool load
    if (q, v) in ((0, 0), (0, 1)):
        return "act"
    if (q, v) == (0, 2):
        return "act" if sup % 2 == 0 else "dve"
    if (q, v) in ((1, 0), (1, 1)):
        return "pool"
    if (q, v) == (1, 2):
        return "pool" if sup < 7 else "dve"
    return "dve"   # (0,3), (1,3), (2,0)


_CACHE = {}
LAST_RESULTS = None


def _build_program():
    import concourse.bass as bass
    import concourse.mybir as mybir
    import concourse.tile as tile
    from concourse import bacc

    f32 = mybir.dt.float32
    f16 = mybir.dt.float16
    f8 = mybir.dt.float8e4
    Alu = mybir.AluOpType
    Act = mybir.ActivationFunctionType

    nc = bacc.Bacc(
        "TRN2",
        target_bir_lowering=False,
        debug=False,
        enable_asserts=False,
        num_devices=NCORES,
    )

    # ---- DRAM I/O ----
    xbT16 = nc.dram_tensor("xbT16", [F, N], f16, kind="ExternalInput").ap()
    xisT16 = nc.dram_tensor("xisT16", [F, NI], f16, kind="ExternalInput").ap()
    wl216 = nc.dram_tensor("wl216", [F, 128], f16, kind="ExternalInput").ap()
    wr16 = nc.dram_tensor("wr16", [F, HC], f16, kind="ExternalInput").ap()
    blp = nc.dram_tensor("blp", [128, 1], f32, kind="ExternalInput").ap()
    brp = nc.dram_tensor("brp", [HC, 1], f32, kind="ExternalInput").ap()
    brpb = nc.dram_tensor("brpb", [HC, 1], f32, kind="ExternalInput").ap()
    attv = nc.dram_tensor("attv", [F, 128], f16, kind="ExternalInput").ap()
    npass = max(1, len(_pass_list()))
    a8stm = nc.dram_tensor("a8stm", [128, npass * 256], f8, kind="ExternalInput").ap()
    mskst = nc.dram_tensor("mskst", [16, 256], f8, kind="ExternalInput").ap()
    adjm8 = nc.dram_tensor("adjm8", [16, 16384], f8, kind="ExternalInput").ap()
    ratiop = nc.dram_tensor("ratiop", [128, 1], f32, kind="ExternalInput").ap()
    attbp = nc.dram_tensor("attbp", [HC, 16], f16, kind="ExternalInput").ap()
    id16m = nc.dram_tensor("id16m", [128, 128], f16, kind="ExternalInput").ap()
    out = nc.dram_tensor("out", [NI, HC], f32, kind="ExternalOutput").ap()

    with tile.TileContext(nc) as tc:
        _body(tc, nc, mybir, f32, f16, f8, Alu, Act,
              xbT16, xisT16, wl216, wr16, blp, brp, brpb, attv, a8stm, mskst,
              adjm8, ratiop, attbp, id16m, out)

    nc.compile()
    return nc


def _body(tc, nc, mybir, f32, f16, f8, Alu, Act,
          xbT16, xisT16, wl216, wr16, blp, brp, brpb, attv, a8stm, mskst,
          adjm8, ratiop, attbp, id16m, out):
    from contextlib import ExitStack
    ctx = ExitStack()
    with ctx:
        consts = ctx.enter_context(tc.tile_pool(name="consts", bufs=1))
        work = ctx.enter_context(tc.tile_pool(name="work", bufs=1))
        rp_pool = ctx.enter_context(tc.tile_pool(name="rp", bufs=16))
        duo_pool = ctx.enter_context(tc.tile_pool(name="duo", bufs=10))
        solo_pool = ctx.enter_context(tc.tile_pool(name="solo", bufs=3))
        sc_pool = ctx.enter_context(tc.tile_pool(name="sc", bufs=4))
        outp = ctx.enter_context(tc.tile_pool(name="outp", bufs=2))
        psg = ctx.enter_context(tc.tile_pool(name="psg", bufs=2, space="PSUM"))
        psb = ctx.enter_context(tc.tile_pool(name="psb", bufs=1, space="PSUM"))
        psa = ctx.enter_context(tc.tile_pool(name="psa", bufs=2, space="PSUM"))

        dma = nc.sync.dma_start
        dma2 = nc.scalar.dma_start      # Act HWDGE queue: output stores
        dmaT = nc.sync.dma_start_transpose

        xT = consts.tile([F, N], f16, tag="xT")       # [f, node]
        xisT = consts.tile([F, NI], f16, tag="xisT")  # [f, dest-slice node]
        wl2_t = consts.tile([F, 128], f16, tag="wl2")
        wr_t = consts.tile([F, HC], f16, tag="wr")
        blp2_t = consts.tile([128, 1], f32, tag="blp2")
        brp_t = consts.tile([HC, 1], f32, tag="brp")
        brpb_t = consts.tile([HC, 1], f32, tag="brpb")  # b_r + bias
        attv_t = consts.tile([F, 128], f16, tag="attv")
        a8st_t = consts.tile([128, max(1, len(_pass_list())) * 256], f8, tag="a8st")
        mskst_t = consts.tile([16, 256], f8, tag="mskst")
        adjm_t = consts.tile([16, 16384], f8, tag="adjm")
        ratio_t = consts.tile([128, 1], f32, tag="ratio")
        attbp_t = consts.tile([HC, 16], f16, tag="attbp")
        id16_t = consts.tile([128, 128], f16, tag="id16")
        dma(xT[:, 0:512], xbT16[:, 0:512])
        dma(wl2_t[:], wl216)
        dma(blp2_t[:], blp)
        dma(ratio_t[:], ratiop)
        dma(xT[:, 512:N], xbT16[:, 512:N])
        dma(xisT[:], xisT16)
        dma(brp_t[:], brp)
        dma(wr_t[:], wr16)
        dma(a8st_t[:], a8stm)
        dma(attv_t[:], attv)
        dma(mskst_t[:], mskst)
        dma(adjm_t[:], adjm8)
        dma(attbp_t[:], attbp)
        dma(brpb_t[:], brpb)
        dma(id16_t[:], id16m)

        # ---------- projections ----------
        xl2T = consts.tile([128, N], f16, tag="xl2T")    # (x@W_l+b_l)^T x2
        xlh2T = consts.tile([128, N], f16, tag="xlh2T")  # xl2T * ratio
        xrT16 = consts.tile([HC, N], f16, tag="xrT16")   # (x@W_r+b_r)^T
        xrsT = consts.tile([HC, NI], f32, tag="xrsT")    # dest-row slice, f32
        pj = psg.tile([128, N], f32, tag="g", name="pj")
        for half in range(2):
            s = slice(half * 512, (half + 1) * 512)
            nc.tensor.matmul(pj[:, s], wl2_t[:], xT[:, s], start=True, stop=True)
        pj3 = psb.tile([HC, NI], f32, tag="b", name="pj3")
        nc.tensor.matmul(pj3[:], wr_t[:], xisT[:], start=True, stop=True)
        for half in range(2):
            s = slice(half * 512, (half + 1) * 512)
            nc.scalar.activation(xl2T[:, s], pj[:, s], Act.Identity,
                                 bias=blp2_t[:, 0:1], scale=1.0)
        nc.vector.tensor_scalar(xlh2T[:], xl2T[:], ratio_t[:, 0:1], 0.0,
                                Alu.mult, Alu.bypass)
        nc.scalar.activation(xrsT[:], pj3[:], Act.Identity,
                             bias=brp_t[:, 0:1], scale=1.0)
        pj2 = psg.tile([HC, N], f32, tag="g", name="pj2")
        for half in range(2):
            s = slice(half * 512, (half + 1) * 512)
            nc.tensor.matmul(pj2[:, s], wr_t[:], xT[:, s], start=True, stop=True)
        nc.scalar.activation(xrT16[:], pj2[:], Act.Identity,
                             bias=brpb_t[:, 0:1], scale=1.0)

        # ---------- xrp: per-pair bias columns [xr[2p] ; xr[2p+1]] ----------
        xrp = consts.tile([128, 128], f32, tag="xrp")
        xrph = consts.tile([128, 128], f32, tag="xrph")  # * ratio
        ev = xrsT[:].rearrange("p (a two) -> p a two", two=2)
        nc.vector.tensor_copy(xrp[0:HC, :], ev[:, :, 0])
        nc.vector.tensor_copy(xrp[HC:128, :], ev[:, :, 1])
        nc.vector.tensor_scalar(xrph[:], xrp[:], ratio_t[:, 0:1], 0.0,
                                Alu.mult, Alu.bypass)

        # ---------- xr_mod build: [j128, k, h, 0:16]=xr*esl, [..,16]=esl ----
        def build_xr_mod():
            psl = psb.tile([16, N], f32, tag="b", name="psl")
            for half in range(2):
                s = slice(half * 512, (half + 1) * 512)
                nc.tensor.matmul(psl[:, s], attbp_t[:], xl2T[0:HC, s],
                                 start=True, stop=True)
            eslT = work.tile([16, N], f16, tag="eslT", name="eslT")
            nc.scalar.activation(eslT[:], psl[:], Act.Exp, scale=0.2)
            xr_nat = work.tile([128, 8 * HC], f16, tag="xrnat", name="xr_nat")
            esln = work.tile([128, 8 * 16], f16, tag="esln", name="esln")
            dmaT(xr_nat[:].rearrange("p (k c) -> p k c", k=8), xrT16[:])
            dmaT(esln[:].rearrange("p (k e) -> p k e", k=8), eslT[:])
            xmv = xr_mod[:].rearrange("p (k h e) -> p k h e", k=8, h=H)
            xnv = xr_nat[:].rearrange("p (k h c) -> p k h c", k=8, h=H)
            rep = esln[:].rearrange("p (k e) -> p k e", k=8)[:, :, 0:H]
            repb = esln[:].rearrange("p (k e one) -> p k e one", k=8, one=1)
            repb = repb[:, :, 0:H, :].broadcast_to([128, 8, H, C])
            nc.vector.tensor_tensor(xmv[:, :, :, 0:C], xnv, repb, Alu.mult)
            nc.vector.tensor_copy(xmv[:, :, :, C], rep)

        xr_mod = consts.tile([128, 8 * 68], f16, tag="xrmod")

        # st_t[ib]: S^T tiles, [j128, k*512 + s4*128 + r], r = PSUM row layout
        st_t = [consts.tile([128, 8 * 512], f16, tag=f"stt{ib}",
                            name=f"stt{ib}") for ib in range(2)]

        # ---------- aggregation ----------
        def aggregate(ib):
            out_f = outp.tile([128, HC], f32, tag="outf", name="outf")
            stv = st_t[ib][:].rearrange("p (k t h) -> p k t h", k=8, h=H)
            agg = psa.tile([128, 4 * 17], f32, tag="a", name="agg")
            for h in range(H):
                for k in range(8):
                    nc.tensor.matmul(agg[:, h * 17:(h + 1) * 17],
                                     stv[:, k, :, h],
                                     xr_mod[:, k * 68 + h * 17: k * 68 + (h + 1) * 17],
                                     start=(k == 0), stop=(k == 7))
            for h in range(H):
                rz = work.tile([128, 1], f32, tag="rz", name="rz")
                nc.vector.reciprocal(rz[:], agg[:, h * 17 + 16:h * 17 + 17])
                nc.vector.tensor_scalar(out_f[:, h * 16:(h + 1) * 16],
                                        agg[:, h * 17:h * 17 + 16], rz[:, 0:1],
                                        None, Alu.mult)
            dma2(out[ib * 128:(ib + 1) * 128, :], out_f[:])

        a8v = a8st_t[:].rearrange("p (ps u c) -> p ps u c",
                                  ps=max(1, len(_pass_list())), u=2)
        mskv = mskst_t[:].rearrange("p (u c) -> p u c", u=2)
        adjv = adjm_t[:].rearrange("p (u S j) -> p u S j", u=2, S=NSUP)

        for sup in range(NSUP):
            ib, s4 = sup // 4, sup % 4
            if sup == 1:
                build_xr_mod()
            if sup == 4:
                aggregate(0)
            gps = psg.tile([128, N], f32, tag="g", name=f"gps{sup}")

            # ---- fp8 production (9 pairs -> 4 duo tiles + 1 solo) ----
            pair_ap = {}
            duos = None
            if MODE != "allf16":
                duos = [duo_pool.tile([128, 2048], f8, tag="duo",
                                      name=f"duo{sup}_{j}") for j in range(4)]
                solo = solo_pool.tile([128, N], f8, tag="solo",
                                      name=f"solo{sup}")
                for j, (pa, pb) in enumerate(FP8_DUOS):
                    for u, (q, v) in enumerate((pa, pb)):
                        pair_ap[(q, v)] = duos[j][:, u * N:(u + 1) * N]
                pair_ap[FP8_SOLO] = solo[:]
            if MODE != "allf16":
                for (q, v) in ALL_FP8:
                    p = sup * 16 + 4 * q + v
                    dst = pair_ap[(q, v)]
                    eng = _fp8_engine(sup, q, v)
                    if eng == "act":
                        nc.scalar.activation(dst, xl2T[:], Act.Relu,
                                             bias=xrph[:, p:p + 1],
                                             scale=ratio_t[:, 0:1])
                    elif eng == "pool":
                        nc.gpsimd.tensor_scalar(dst, xlh2T[:], xrph[:, p:p + 1],
                                                0.0, Alu.add, Alu.max)
                    else:
                        nc.vector.tensor_scalar(dst, xlh2T[:], xrph[:, p:p + 1],
                                                0.0, Alu.add, Alu.max)

            # ---- f16 production (DVE) ----
            f16_pairs = ([(q, v) for q in range(4) for v in range(4)]
                         if MODE == "allf16" else F16_PAIRS)
            rps = {}
            for (q, v) in f16_pairs:
                p = sup * 16 + 4 * q + v
                rp = rp_pool.tile([128, N], f16, tag="rp")
                nc.vector.tensor_scalar(rp[:], xl2T[:], xrp[:, p:p + 1],
                                        0.0, Alu.add, Alu.max)
                rps[q, v] = rp

            # ---- score matmuls ----
            passes = _pass_list()
            for half in range(2):
                s = slice(half * 512, (half + 1) * 512)
                first = True
                for ps, (pa, pb) in enumerate(passes):
                    if pb is not None:      # duo DR pass
                        mv = duos[ps][:].rearrange("p (u j) -> p u j", u=2)
                        nc.tensor.matmul(
                            gps[:, s], a8v[:, ps, :, :], mv[:, :, s],
                            start=first, stop=False,
                            perf_mode=mybir.MatmulPerfMode.DoubleRow,
                            tile_position=(0, 0), skip_group_check=True)
                    elif MODE == "plainfp8":
                        nc.tensor.matmul(
                            gps[:, s], a8v[:, ps, 0, :], pair_ap[pa][:, s],
                            start=first, stop=False,
                            tile_position=(0, 0), skip_group_check=True)
                    else:                    # solo DR pass (u1 stationary = 0)
                        mv1 = pair_ap[pa].rearrange("p (one j) -> p one j",
                                                    one=1)
                        nc.tensor.matmul(
                            gps[:, s], a8v[:, ps, :, :],
                            mv1[:, :, s].broadcast_to([128, 2, 512]),
                            start=first, stop=False,
                            perf_mode=mybir.MatmulPerfMode.DoubleRow,
                            tile_position=(0, 0), skip_group_check=True)
                    first = False
                nc.tensor.matmul(
                    gps[:, s], mskv[:, :, :], adjv[:, :, sup, s],
                    start=first, stop=(MODE == "allf16" and not f16_pairs),
                    perf_mode=mybir.MatmulPerfMode.DoubleRow,
                    tile_position=(0, 0), skip_group_check=True)
                for (q, v) in f16_pairs:
                    nc.tensor.matmul(
                        gps[32 * q:32 * q + 32, s],
                        attv_t[:, 32 * v:32 * v + 32],
                        rps[q, v][:, s],
                        start=False, stop=((q, v) == f16_pairs[-1]),
                        tile_position=(0, 32 * q),
                        skip_group_check=True,
                    )

            # ---- exp + scatter to S^T layout ----
            dstv = st_t[ib][:].rearrange("p (k s r) -> p k s r", k=8, s=4)
            for half in range(2):
                s = slice(half * 512, (half + 1) * 512)
                scomp = sc_pool.tile([128, 512], f16, tag="scomp",
                                     name=f"sc{sup}_{half}")
                nc.scalar.activation(scomp[:], gps[:, s], Act.Exp)
                if sup == NSUP - 1:
                    # tail: PE transpose (short latency) instead of DMA xbar
                    for k in range(half * 4, half * 4 + 4):
                        pt = psa.tile([128, 128], f16, tag="a", name="pt")
                        nc.tensor.transpose(
                            pt[:], scomp[:, (k - half * 4) * 128:
                                         (k - half * 4 + 1) * 128], id16_t[:])
                        nc.vector.tensor_copy(dstv[:, k, s4, :], pt[:])
                else:
                    dmaT(dstv[:, half * 4:(half + 1) * 4, s4, :], scomp[:])

        aggregate(1)


def _get_program():
    if MODE not in _CACHE:
        _CACHE[MODE] = _build_program()
    return _CACHE[MODE]


def kernel(x, adj, W_l, b_l, W_r, b_r, att, bias):
    global LAST_RESULTS
    import ml_dtypes
    from concourse.bass_utils import run_bass_kernel_spmd

    x = np.ascontiguousarray(np.asarray(x, dtype=np.float32))
    adj = np.ascontiguousarray(np.asarray(adj, dtype=np.float32))
    W_l = np.asarray(W_l, dtype=np.float32)
    b_l = np.asarray(b_l, dtype=np.float32)
    W_r = np.asarray(W_r, dtype=np.float32)
    b_r = np.asarray(b_r, dtype=np.float32)
    att = np.asarray(att, dtype=np.float32)
    bias = np.asarray(bias, dtype=np.float32)

    # ---- host-side constant prep ----
    # fp16 att stationary for the fp16 bands (q=2 v>=1, q=3)
    attv = np.zeros((F, 128), np.float32)
    for v in range(4):
        for d in range(2):
            for h in range(H):
                col = 32 * v + 8 * v + 4 * d + h
                attv[d * HC + h * C:d * HC + (h + 1) * C, col] = 0.8 * att[h]
    attv = attv.astype(np.float16)

    # fp8 att stationaries for the DoubleRow duo/solo passes.
    att8 = (0.8 * att.astype(np.float32)).astype(ml_dtypes.float8_e4m3)
    att8f = att8.astype(np.float32)
    # ratio[hc] = 0.8*att/fp8(0.8*att) (1.0 where att==0), dup'd over d
    with np.errstate(divide="ignore", invalid="ignore"):
        rat = np.where(att8f != 0.0, 0.8 * att / att8f, 1.0)
    ratio = np.concatenate([rat.reshape(HC), rat.reshape(HC)])
    ratio = ratio.reshape(128, 1).astype(np.float32)

    passes = _pass_list()
    npass = max(1, len(passes))
    a8st = np.zeros((128, npass, 2, 128), np.float32)
    for ps, (pa, pb) in enumerate(passes):
        for u, pair in enumerate((pa, pb)):
            if pair is None:
                continue
            q, v = pair
            for d in range(2):
                for h in range(H):
                    col = 32 * q + 8 * v + 4 * d + h
                    a8st[d * HC + h * C:d * HC + (h + 1) * C, ps, u, col] = att8f[h]
    a8stm = a8st.reshape(128, npass * 256).astype(ml_dtypes.float8_e4m3)

    # PSUM row r = 32q+8v+4d+h  <->  dest-in-super ld = 8q+2v+d
    rowld = np.zeros(128, np.int64)
    for q in range(4):
        for v in range(4):
            for d in range(2):
                for h in range(H):
                    rowld[32 * q + 8 * v + 4 * d + h] = 8 * q + 2 * v + d
    # mask stationary: mskst[ld%16, ld//16, r] = 1 where rowld[r]=ld
    mskst = np.zeros((16, 2, 128), np.float32)
    for r in range(128):
        ld = rowld[r]
        mskst[ld % 16, ld // 16, r] = 1.0
    mskst = mskst.reshape(16, 256).astype(ml_dtypes.float8_e4m3)

    attbp = np.zeros((HC, 16), np.float32)
    for h in range(H):
        attbp[h * C:(h + 1) * C, h] = att[h]
    attbp = attbp.astype(np.float16)
    blp = np.concatenate([b_l, b_l]).reshape(128, 1).astype(np.float32)
    brp = b_r.reshape(HC, 1).astype(np.float32).copy()
    brpb = (b_r + bias).reshape(HC, 1).astype(np.float32).copy()
    wl216 = np.concatenate([W_l, W_l], axis=1).astype(np.float16)
    wr16 = W_r.astype(np.float16).copy()
    id16 = np.eye(128, dtype=np.float16)

    in_maps = []
    for core in range(NCORES):
        b, blk = core // 4, core % 4
        i0 = blk * NI
        adjsl = adj[b, i0:i0 + NI, :].copy()
        adjsl[np.arange(NI), i0 + np.arange(NI)] = 1.0   # self loops
        # adjm8[k, u, sup, j] = -15*(1-adj[32*sup + k + 16u, j])
        a4 = adjsl.reshape(NSUP, 2, 16, N)   # [sup, u, k, j]
        adjm = -15.0 * (1.0 - a4.transpose(2, 1, 0, 3))   # [k, u, sup, j]
        adjm = np.ascontiguousarray(adjm).reshape(16, 16384)
        adjm = adjm.astype(ml_dtypes.float8_e4m3)
        in_maps.append({
            "xbT16": np.ascontiguousarray(x[b].T).astype(np.float16),
            "xisT16": np.ascontiguousarray(x[b, i0:i0 + NI].T).astype(np.float16),
            "wl216": wl216, "wr16": wr16, "blp": blp, "brp": brp,
            "brpb": brpb, "attv": attv, "a8stm": a8stm, "mskst": mskst,
            "adjm8": adjm, "ratiop": ratio, "attbp": attbp, "id16m": id16,
        })

    nc = _get_program()
    res = run_bass_kernel_spmd(nc, in_maps, core_ids=list(range(NCORES)))
    LAST_RESULTS = res
    outp = np.zeros((B, N, HC), np.float32)
    for core in range(NCORES):
        b, blk = core // 4, core % 4
        outp[b, blk * NI:(blk + 1) * NI, :] = res.results[core]["out"]
    return outp


# revision 16
# speedup vs baseline: 1.3935x; 1.3935x over previous
# DenseGATv2Conv Trainium2 kernel (v4).
#
# Math (per batch b):
#   xl = x @ W_l + b_l ; xr = x @ W_r + b_r            [N, H*C]
#   alpha[i,j,h] = sum_c att[h,c] * leaky_relu(xl[j,hc] + xr[i,hc], 0.2)
#   S = softmax_j(alpha masked by adj(+self loops))
#   out[i,hc] = sum_j S[i,j,h] * xr[j,hc] + bias
#
# Identities used on device:
#   leaky_relu(z) = 0.2*z + 0.8*relu(z)
#   alpha[i,j,h] = 0.2*sl[j,h] + 0.2*sr[i,h] + 0.8*sum_c att[h,c]*relu(xl[j,hc]+xr[i,hc])
# exp(0.2*sr[i,h]) cancels in the softmax; exp(0.2*sl[j,h]) (= esl) is folded
# multiplicatively into the aggregation operand; the output bias is folded
# into the aggregation operand too, via (num + bias*den)/den.
#
# v4 structure (per core: 8 supers of 32 dest rows = 16 dest-row pairs):
#  * 8 pairs/super (q=0,1) in fp8: relu data produced directly in fp8e4m3
#    (production split DVE/Act/GpSimd), consumed by DoubleRow matmuls
#    packing TWO pairs per pass (0.5 PE cycles/row).  fp8 rounding of
#    0.8*att is exactly compensated by pre-scaling the relu production with
#    ratio[hc] = 0.8*att/fp8(0.8*att) (host-folded into xlh/xrph), leaving
#    only the relu-value e4m3 noise (~1.5e-2 overall rel, gate is 2e-2).
#  * 8 pairs/super (q=2,3) in f16 (DVE production + banded f16 matmuls).
#  * adjacency mask = -15 additive bias via one fp8 DoubleRow matmul per
#    half (moving = -15*(1-adj) fp8, stationary = dest-row selector).
#  * the small O(N*F*HC) projections (xl, xr, esl) are host-precomputed;
#    the device runs only the O(N^2) score/softmax/aggregation pipeline.
#    Inputs are packed into 6 load DMAs (HWDGE enqueue is ~0.6us each).
#
# Sharding: 8 cores = (batch b in 0..1) x (4 blocks of 256 destination rows).

import numpy as np

B, N, F, H, C = 2, 1024, 128, 4, 16
HC = H * C
NCORES = 8
NI = 256          # destination rows per core
NSUP = 8          # supers of 16 pairs (32 dest rows) each

# fp8 duo passes: [(q,v),(q,v+1)] share one DoubleRow matmul per half.
FP8_DUOS = [((0, 0), (0, 1)), ((0, 2), (0, 3)),
            ((1, 0), (1, 1)), ((1, 2), (1, 3))]
F16_PAIRS = [(2, 0), (2, 1), (2, 2), (2, 3), (3, 0), (3, 1), (3, 2), (3, 3)]
ALL_FP8 = [p for duo in FP8_DUOS for p in duo]


def _fp8_engine(sup, q, v):
    # production engine per fp8 pair, balancing DVE/Act/Pool busy time
    if (q, v) in ((0, 0), (0, 1), (0, 2)):
        return "act"                                   # 24
    if (q, v) == (0, 3):
        return "act" if sup == 0 else "dve"            # 1 + 7
    if (q, v) in ((1, 0), (1, 1), (1, 2)):
        return "pool"                                  # 24
    return "dve"                                       # (1,3): 8


_CACHE = {}
LAST_RESULTS = None


def _build_program():
    import concourse.bass as bass
    import concourse.mybir as mybir
    import concourse.tile as tile
    from concourse import bacc

    f32 = mybir.dt.float32
    f16 = mybir.dt.float16
    f8 = mybir.dt.float8e4

    nc = bacc.Bacc(
        "TRN2",
        target_bir_lowering=False,
        debug=False,
        enable_asserts=False,
        num_devices=NCORES,
    )

    # ---- DRAM I/O (packed to minimize DMA count) ----
    # xlpk: [128, 2048] f16 = xl2T | xlh2T
    xlpk = nc.dram_tensor("xlpk", [128, 2 * N], f16, kind="ExternalInput").ap()
    # xrpk: [80, 1024] f16 = xrT16 (rows 0:64) | eslT (rows 64:80)
    xrpk = nc.dram_tensor("xrpk", [80, N], f16, kind="ExternalInput").ap()
    # xrpp: [128, 256] f32 = xrp | xrph  (per-pair bias columns)
    xrpp = nc.dram_tensor("xrpp", [128, 256], f32, kind="ExternalInput").ap()
    # avid: [128, 256] f16 = attv | id16
    avid = nc.dram_tensor("avid", [128, 256], f16, kind="ExternalInput").ap()
    # a8pk: [128, 1280] f8 = a8st (4*2*128) | mskst (rows 0:16, cols 1024:1280)
    a8pk = nc.dram_tensor("a8pk", [128, 1280], f8, kind="ExternalInput").ap()
    adjm8 = nc.dram_tensor("adjm8", [16, 16384], f8, kind="ExternalInput").ap()
    out = nc.dram_tensor("out", [NI, HC], f32, kind="ExternalOutput").ap()

    with tile.TileContext(nc) as tc:
        _body(tc, nc, mybir, f32, f16, f8,
              xlpk, xrpk, xrpp, avid, a8pk, adjm8, out)

    nc.compile()
    return nc


def _body(tc, nc, mybir, f32, f16, f8, xlpk, xrpk, xrpp, avid, a8pk, adjm8,
          out):
    from contextlib import ExitStack
    Alu = mybir.AluOpType
    Act = mybir.ActivationFunctionType
    ctx = ExitStack()
    with ctx:
        consts = ctx.enter_context(tc.tile_pool(name="consts", bufs=1))
        work = ctx.enter_context(tc.tile_pool(name="work", bufs=1))
        rp_pool = ctx.enter_context(tc.tile_pool(name="rp", bufs=18))
        duo_pool = ctx.enter_context(tc.tile_pool(name="duo", bufs=10))
        sc_pool = ctx.enter_context(tc.tile_pool(name="sc", bufs=3))
        outp = ctx.enter_context(tc.tile_pool(name="outp", bufs=2))
        psg = ctx.enter_context(tc.tile_pool(name="psg", bufs=2, space="PSUM"))
        psa = ctx.enter_context(tc.tile_pool(name="psa", bufs=2, space="PSUM"))

        dma = nc.sync.dma_start
        dma2 = nc.scalar.dma_start      # Act HWDGE queue: output stores
        dmaT = nc.sync.dma_start_transpose

        xlt = consts.tile([128, 2 * N], f16, tag="xlt")    # xl2T | xlh2T
        xrt = consts.tile([80, N], f16, tag="xrt")         # xrT16 | eslT
        xrpp_t = consts.tile([128, 256], f32, tag="xrpp")  # xrp | xrph
        avid_t = consts.tile([128, 256], f16, tag="avid")  # attv | id16
        a8pk_t = consts.tile([128, 1280], f8, tag="a8pk")
        adjm_t = consts.tile([16, 16384], f8, tag="adjm")
        dma(xlt[:], xlpk)
        dma(xrpp_t[:], xrpp)
        dma(a8pk_t[:], a8pk)
        dma(adjm_t[:], adjm8)
        dma(avid_t[:], avid)
        dma(xrt[:], xrpk)

        xl2T = xlt[:, 0:N]
        xlh2T = xlt[:, N:2 * N]
        xrT16 = xrt[0:HC, :]
        eslT = xrt[HC:HC + 16, :]
        xrp = xrpp_t[:, 0:128]
        xrph = xrpp_t[:, 128:256]
        attv_t = avid_t[:, 0:128]
        id16_t = avid_t[:, 128:256]
        a8v = a8pk_t[:, 0:1024].rearrange("p (ps u c) -> p ps u c", ps=4, u=2)
        mskv = a8pk_t[0:16, 1024:1280].rearrange("p (u c) -> p u c", u=2)
        adjv = adjm_t[:].rearrange("p (u S j) -> p u S j", u=2, S=NSUP)

        # ---------- xr_mod: [j128, k, h, 0:16]=xr*esl, [..,16]=esl ----------
        xr_mod = consts.tile([128, 8 * 68], f16, tag="xrmod")

        def build_xr_mod():
            xr_nat = work.tile([128, 8 * HC], f16, tag="xrnat", name="xr_nat")
            esln = work.tile([128, 8 * 16], f16, tag="esln", name="esln")
            dmaT(xr_nat[:].rearrange("p (k c) -> p k c", k=8), xrT16)
            dmaT(esln[:].rearrange("p (k e) -> p k e", k=8), eslT)
            xmv = xr_mod[:].rearrange("p (k h e) -> p k h e", k=8, h=H)
            xnv = xr_nat[:].rearrange("p (k h c) -> p k h c", k=8, h=H)
            rep = esln[:].rearrange("p (k e) -> p k e", k=8)[:, :, 0:H]
            repb = esln[:].rearrange("p (k e one) -> p k e one", k=8, one=1)
            repb = repb[:, :, 0:H, :].broadcast_to([128, 8, H, C])
            nc.vector.tensor_tensor(xmv[:, :, :, 0:C], xnv, repb, Alu.mult)
            nc.vector.tensor_copy(xmv[:, :, :, C], rep)

        # st_t[ib]: S^T tiles, [j128, k*512 + s4*128 + r], r = PSUM row layout
        st_t = [consts.tile([128, 8 * 512], f16, tag=f"stt{ib}",
                            name=f"stt{ib}") for ib in range(2)]

        # ---------- aggregation ----------
        def aggregate(ib):
            out_f = outp.tile([128, HC], f32, tag="outf", name="outf")
            stv = st_t[ib][:].rearrange("p (k t h) -> p k t h", k=8, h=H)
            agg = psa.tile([128, 4 * 17], f32, tag="a", name="agg")
            for h in range(H):
                for k in range(8):
                    nc.tensor.matmul(agg[:, h * 17:(h + 1) * 17],
                                     stv[:, k, :, h],
                                     xr_mod[:, k * 68 + h * 17: k * 68 + (h + 1) * 17],
                                     start=(k == 0), stop=(k == 7))
            for h in range(H):
                rz = work.tile([128, 1], f32, tag="rz", name="rz")
                nc.vector.reciprocal(rz[:], agg[:, h * 17 + 16:h * 17 + 17])
                nc.vector.tensor_scalar(out_f[:, h * 16:(h + 1) * 16],
                                        agg[:, h * 17:h * 17 + 16], rz[:, 0:1],
                                        None, Alu.mult)
            dma2(out[ib * 128:(ib + 1) * 128, :], out_f[:])

        for sup in range(NSUP):
            ib, s4 = sup // 4, sup % 4
            if sup == 1:
                build_xr_mod()
            if sup == 4:
                aggregate(0)
            gps = psg.tile([128, N], f32, tag="g", name=f"gps{sup}")

            # ---- fp8 production (8 pairs -> 4 duo tiles) ----
            duos = [duo_pool.tile([128, 2048], f8, tag="duo",
                                  name=f"duo{sup}_{j}") for j in range(4)]
            for j, (pa, pb) in enumerate(FP8_DUOS):
                for u, (q, v) in enumerate((pa, pb)):
                    p = sup * 16 + 4 * q + v
                    dst = duos[j][:, u * N:(u + 1) * N]
                    eng = _fp8_engine(sup, q, v)
                    if eng == "act":
                        nc.scalar.activation(dst, xlh2T, Act.Relu,
                                             bias=xrph[:, p:p + 1], scale=1.0)
                    elif eng == "pool":
                        nc.gpsimd.tensor_scalar(dst, xlh2T, xrph[:, p:p + 1],
                                                0.0, Alu.add, Alu.max)
                    else:
                        nc.vector.tensor_scalar(dst, xlh2T, xrph[:, p:p + 1],
                                                0.0, Alu.add, Alu.max)

            # ---- f16 production (8 pairs, DVE) ----
            rps = {}
            for (q, v) in F16_PAIRS:
                p = sup * 16 + 4 * q + v
                rp = rp_pool.tile([128, N], f16, tag="rp")
                nc.vector.tensor_scalar(rp[:], xl2T, xrp[:, p:p + 1],
                                        0.0, Alu.add, Alu.max)
                rps[q, v] = rp

            # ---- score matmuls ----
            for half in range(2):
                s = slice(half * 512, (half + 1) * 512)
                for j in range(4):
                    mv = duos[j][:].rearrange("p (u j) -> p u j", u=2)
                    nc.tensor.matmul(
                        gps[:, s], a8v[:, j, :, :], mv[:, :, s],
                        start=(j == 0), stop=False,
                        perf_mode=mybir.MatmulPerfMode.DoubleRow,
                        tile_position=(0, 0), skip_group_check=True)
                nc.tensor.matmul(
                    gps[:, s], mskv[:, :, :], adjv[:, :, sup, s],
                    start=False, stop=False,
                    perf_mode=mybir.MatmulPerfMode.DoubleRow,
                    tile_position=(0, 0), skip_group_check=True)
                for (q, v) in F16_PAIRS:
                    nc.tensor.matmul(
                        gps[32 * q:32 * q + 32, s],
                        attv_t[:, 32 * v:32 * v + 32],
                        rps[q, v][:, s],
                        start=False, stop=((q, v) == F16_PAIRS[-1]),
                        tile_position=(0, 32 * q),
                        skip_group_check=True,
                    )

            # ---- exp + scatter to S^T layout ----
            dstv = st_t[ib][:].rearrange("p (k s r) -> p k s r", k=8, s=4)
            scomp = sc_pool.tile([128, N], f16, tag="scomp", name=f"sc{sup}")
            for half in range(2):
                s = slice(half * 512, (half + 1) * 512)
                nc.scalar.activation(scomp[:, s], gps[:, s], Act.Exp)
                if sup == NSUP - 1:
                    # tail: PE transpose (short latency) instead of DMA xbar
                    for k in range(half * 4, half * 4 + 4):
                        pt = psa.tile([128, 128], f16, tag="a", name="pt")
                        nc.tensor.transpose(pt[:],
                                            scomp[:, k * 128:(k + 1) * 128],
                                            id16_t)
                        nc.vector.tensor_copy(dstv[:, k, s4, :], pt[:])
                else:
                    dmaT(dstv[:, half * 4:(half + 1) * 4, s4, :], scomp[:, s])

        aggregate(1)


def _get_program():
    if "nc" not in _CACHE:
        _CACHE["nc"] = _build_program()
    return _CACHE["nc"]


def kernel(x, adj, W_l, b_l, W_r, b_r, att, bias):
    global LAST_RESULTS
    import ml_dtypes
    from concourse.bass_utils import run_bass_kernel_spmd

    x = np.ascontiguousarray(np.asarray(x, dtype=np.float32))
    adj = np.ascontiguousarray(np.asarray(adj, dtype=np.float32))
    W_l = np.asarray(W_l, dtype=np.float32)
    b_l = np.asarray(b_l, dtype=np.float32)
    W_r = np.asarray(W_r, dtype=np.float32)
    b_r = np.asarray(b_r, dtype=np.float32)
    att = np.asarray(att, dtype=np.float32)
    bias = np.asarray(bias, dtype=np.float32)

    # ---- host-side projections (O(N*F*HC), ~0.1% of the N^2 device work) --
    attf = att.reshape(HC)
    att8f = (0.8 * attf).astype(ml_dtypes.float8_e4m3).astype(np.float32)
    with np.errstate(divide="ignore", invalid="ignore"):
        rat = np.where(att8f != 0.0, 0.8 * attf / att8f, 1.0)
    rat2 = np.concatenate([rat, rat])                    # [128] (d, hc)

    # fp16 att stationary for the f16 bands + id16
    attv = np.zeros((F, 128), np.float32)
    for v in range(4):
        for d in range(2):
            for h in range(H):
                col = 32 * v + 8 * v + 4 * d + h
                attv[d * HC + h * C:d * HC + (h + 1) * C, col] = 0.8 * att[h]
    avid = np.concatenate([attv, np.eye(128, dtype=np.float32)], axis=1)
    avid = avid.astype(np.float16)

    # fp8 stationaries: 4 duo passes + mask selector, packed
    a8st = np.zeros((128, 4, 2, 128), np.float32)
    for ps, (pa, pb) in enumerate(FP8_DUOS):
        for u, (q, v) in enumerate((pa, pb)):
            for d in range(2):
                for h in range(H):
                    col = 32 * q + 8 * v + 4 * d + h
                    a8st[d * HC + h * C:d * HC + (h + 1) * C, ps, u, col] = \
                        att8f[h * C:(h + 1) * C]
    rowld = np.zeros(128, np.int64)
    for q in range(4):
        for v in range(4):
            for d in range(2):
                for h in range(H):
                    rowld[32 * q + 8 * v + 4 * d + h] = 8 * q + 2 * v + d
    mskst = np.zeros((16, 2, 128), np.float32)
    for r in range(128):
        ld = rowld[r]
        mskst[ld % 16, ld // 16, r] = 1.0
    a8pk = np.zeros((128, 1280), np.float32)
    a8pk[:, 0:1024] = a8st.reshape(128, 1024)
    a8pk[0:16, 1024:1280] = mskst.reshape(16, 256)
    a8pk = a8pk.astype(ml_dtypes.float8_e4m3)

    per_b = {}
    for b in range(B):
        xb = x[b]
        xl = (xb @ W_l + b_l).astype(np.float32)         # [N, HC]
        xr = (xb @ W_r + b_r).astype(np.float32)
        xl2 = np.concatenate([xl, xl], axis=1)           # [N, 128]
        xlpk = np.concatenate([xl2.T, (xl2 * rat2).T], axis=1)  # [128, 2N]
        # xrT16 folds output bias via (num + bias*den)/den
        xrT16 = (xr + bias).T                            # [HC, N]
        sl = (xl.reshape(N, H, C) * att[None]).sum(-1)   # [N, H]
        eslT16 = np.zeros((16, N), np.float32)
        eslT16[0:H] = np.exp(0.2 * sl).T
        xrpk = np.concatenate([xrT16, eslT16], axis=0).astype(np.float16)
        per_b[b] = (np.ascontiguousarray(xlpk).astype(np.float16),
                    np.ascontiguousarray(xrpk), xr)

    in_maps = []
    for core in range(NCORES):
        b, blk = core // 4, core % 4
        i0 = blk * NI
        xlpk16, xrpk, xr = per_b[b]
        # per-pair bias columns: xrp[d*HC+hc, a] = xr[2a+d, hc]
        xrs = xr[i0:i0 + NI]                             # [NI, HC]
        xrp = np.zeros((128, 128), np.float32)
        xrp[0:HC] = xrs[0::2].T
        xrp[HC:128] = xrs[1::2].T
        xrph = xrp * rat2[:, None]
        xrpp = np.concatenate([xrp, xrph], axis=1)       # [128, 256]

        adjsl = adj[b, i0:i0 + NI, :].copy()
        adjsl[np.arange(NI), i0 + np.arange(NI)] = 1.0   # self loops
        a4 = adjsl.reshape(NSUP, 2, 16, N)               # [sup, u, k, j]
        adjm = -15.0 * (1.0 - a4.transpose(2, 1, 0, 3))  # [k, u, sup, j]
        adjm = np.ascontiguousarray(adjm).reshape(16, 16384)
        in_maps.append({
            "xlpk": xlpk16, "xrpk": xrpk,
            "xrpp": np.ascontiguousarray(xrpp),
            "avid": avid, "a8pk": a8pk,
            "adjm8": adjm.astype(ml_dtypes.float8_e4m3),
        })

    nc = _get_program()
    res = run_bass_kernel_spmd(nc, in_maps, core_ids=list(range(NCORES)))
    LAST_RESULTS = res
    outp = np.zeros((B, N, HC), np.float32)
    for core in range(NCORES):
        b, blk = core // 4, core % 4
        outp[b, blk * NI:(blk + 1) * NI, :] = res.results[core]["out"]
    return outp
